# revision 20
# baseline (speedup 1.0000x reference)
"""Multi-head linear cross-attention Trainium2 kernel (8-core SPMD).

Math (reference):
    q    = fm(query @ Wq.T)        fm(x) = elu(x)+1 = max(x+1, min(exp(x), 1))
    gate = sigmoid(query @ Wg.T + bg)
    k, v = split(kv @ Wkv.T); k = fm(k)
    kvs  = k^T v per (b, h)   [hd, hd];  ksum = k^T 1  [hd]
    attn = (q @ kvs) / (q . ksum + eps)
    out  = (attn * gate) @ Wo.T

Sharding: kv-side split along S (each core takes S/8 rows of every batch,
computes partial [k^T v | k^T 1] summaries, AllReduce over the 8 cores);
q-side split along L (each core takes L/8 rows of every batch).

Precision: the q, gate and k projections run as fp8e4 DoubleRow
matmuls (2 fp8 contractions per PE cell per cycle -> up to 2x matmul
rate). Their weights are host-scaled by WS=64 to keep fp8 operands out
of the subnormal range; the compensation folds away algebraically:
  - the q/k feature maps are computed at 64x scale (exp(x/64 + ln64)),
    and the 64x factors cancel exactly in attn = (q.kvs)/(q.ksum).
  - gate uses activation scale=1/64 before the sigmoid.
The v projection, summaries, attention and output projection stay bf16:
their quantization error would hit the output directly (measured ~3.8e-2
each in simulation vs the 1.2e-2 this mix achieves).

Phase A streams Wkv once in 4 head-groups (k-block + v-block per group),
holding kvT resident (fp8 copy for the k matmuls, bf16 for v); summaries
are computed per (b, m)-tile right after projection (with [v | 1] fused
so k^T*1 rides the same matmul) and each group's summaries are
AllReduced as soon as they are ready, so phase B's head loop never waits
on a collective. den is computed pre-broadcast with a ksum-replicated
stationary matmul.

Layout: all activations are feature-major ([d, token]) on device; the
host pre-transposes/casts inputs and re-transposes the fp32 output.
"""

import numpy as np
import ml_dtypes

import concourse.bacc as bacc
import concourse.mybir as mybir
import concourse.tile as tile
from concourse.bass_utils import run_bass_kernel_spmd

BF16 = mybir.dt.bfloat16
F8 = mybir.dt.float8e4
F32 = mybir.dt.float32
AF = mybir.ActivationFunctionType
ALU = mybir.AluOpType
DR = mybir.MatmulPerfMode.DoubleRow

N_CORES = 8
WS = 64.0          # fp8 weight scale
LN_WS = float(np.log(WS))


def build_module(B=4, L=4096, S=4096, D=2048, H=16, n_cores=N_CORES,
                 use_collective=True):
    """Emit the SPMD bass module. Requires D // H == 128."""
    HD = D // H
    assert HD == 128
    KB = D // 128            # d_in 128-blocks
    KH = KB // 2             # d_in 256-blocks (fp8 DoubleRow steps)
    S_LOC = S // n_cores     # s rows per batch per core
    MB = S_LOC // 128        # s-tiles per batch
    LT = L // n_cores        # l cols per batch per core (free dim of q-side matmuls)
    NG = 4                   # head groups for wkv streaming
    HPG = H // NG            # heads per group
    WG = HPG * HD            # wkv columns per group block (512)
    assert S_LOC % 128 == 0 and D % 128 == 0 and H % NG == 0
    assert LT <= 512 and WG <= 512

    nc = bacc.Bacc("TRN2", target_bir_lowering=False, debug=False,
                   num_devices=n_cores)

    qT = nc.dram_tensor("qT", [D, B * LT], F8, kind="ExternalInput")
    kvT = nc.dram_tensor("kvT", [D, B * S_LOC], BF16, kind="ExternalInput")
    kvT8 = nc.dram_tensor("kvT8", [D, B * S_LOC], F8, kind="ExternalInput")
    wq_t = nc.dram_tensor("wq_t", [D, D], F8, kind="ExternalInput")
    wg_t = nc.dram_tensor("wg_t", [D, D], F8, kind="ExternalInput")
    wkvk_t = nc.dram_tensor("wkvk_t", [D, D], F8, kind="ExternalInput")
    wkvv_t = nc.dram_tensor("wkvv_t", [D, D], BF16, kind="ExternalInput")
    wo_t = nc.dram_tensor("wo_t", [D, D], BF16, kind="ExternalInput")
    bg_d = nc.dram_tensor("bg_d", [D, 1], F32, kind="ExternalInput")
    outT = nc.dram_tensor("outT", [D, B * LT], BF16, kind="ExternalOutput")

    qT_r = qT.ap().rearrange("(k p) l -> p k l", p=128)
    kvT_r = kvT.ap().rearrange("(k p) s -> p k s", p=128)
    kvT8_r = kvT8.ap().rearrange("(k p) s -> p k s", p=128)
    wq_r = wq_t.ap().rearrange("(k p) f -> p k f", p=128)
    wg_r = wg_t.ap().rearrange("(k p) f -> p k f", p=128)
    wkvk_r = wkvk_t.ap().rearrange("(k p) f -> p k f", p=128)
    wkvv_r = wkvv_t.ap().rearrange("(k p) f -> p k f", p=128)
    wo_r = wo_t.ap().rearrange("(k p) f -> p k f", p=128)

    with nc.allow_low_precision(reason="bf16/fp8 matmul kernel"), \
         tile.TileContext(nc) as tc:
        with tc.tile_pool(name="const", bufs=1) as constp, \
             tc.tile_pool(name="qt", bufs=1) as qtp, \
             tc.tile_pool(name="pre", bufs=1) as prep, \
             tc.tile_pool(name="dram", bufs=1, space="DRAM") as dr:

            ones_bf = constp.tile([128, 128], BF16)
            nc.vector.memset(ones_bf[:], 1.0)
            lnws_sb = constp.tile([128, 1], F32)
            nc.vector.memset(lnws_sb[:], LN_WS)

            qT_all = qtp.tile([128, KB, B * LT], F8)

            ar_in = [dr.tile([B, 128, HPG, 129], F32, name=f"ar_in{p}")
                     for p in range(NG)]
            ar_out = [dr.tile([B, 128, HPG, 129], F32, addr_space="Shared",
                              name=f"ar_out{p}") for p in range(NG)]

            # ---------------- Phase A: kv projection + summaries ----------
            with tc.tile_pool(name="kvt", bufs=1) as kvtp, \
                 tc.tile_pool(name="sbA", bufs=2) as sba, \
                 tc.tile_pool(name="psA", bufs=2, space="PSUM") as psa:
                kvT_all = kvtp.tile([128, KB, B * S_LOC], BF16)
                kvT8_all = kvtp.tile([128, KB, B * S_LOC], F8)
                wkv_vs = {}

                def load_wkv_v(p):
                    wkv_vs[p] = sba.tile([128, KB, WG], BF16, tag="wkv_v",
                                         bufs=2, name=f"wkvv{p}")
                    nc.sync.dma_start(wkv_vs[p][:],
                                      wkvv_r[:, :, p * WG:(p + 1) * WG])

                for p in range(NG):
                    wkv_k = sba.tile([128, KB, WG], F8, tag="wkv_k", bufs=2,
                                     name=f"wkvk{p}")
                    nc.sync.dma_start(wkv_k[:],
                                      wkvk_r[:, :, p * WG:(p + 1) * WG])
                    if p == 0:
                        # dependency-ordered preload: everything the first
                        # (b=0, m=0) tile needs first, the bulk afterwards
                        nc.sync.dma_start(kvT8_all[:, :, 0:128],
                                          kvT8_r[:, :, 0:128])
                        load_wkv_v(0)
                        nc.sync.dma_start(kvT_all[:, :, 0:128],
                                          kvT_r[:, :, 0:128])
                        if S_LOC > 128:
                            nc.sync.dma_start(kvT8_all[:, :, 128:S_LOC],
                                              kvT8_r[:, :, 128:S_LOC])
                            nc.sync.dma_start(kvT_all[:, :, 128:S_LOC],
                                              kvT_r[:, :, 128:S_LOC])
                        for b in range(1, B):
                            sl = slice(b * S_LOC, (b + 1) * S_LOC)
                            nc.sync.dma_start(kvT8_all[:, :, sl], kvT8_r[:, :, sl])
                            nc.sync.dma_start(kvT_all[:, :, sl], kvT_r[:, :, sl])
                    if p + 1 < NG:
                        load_wkv_v(p + 1)  # prefetch next group's v-weights
                    wkv_v = wkv_vs[p]
                    if p == 2:
                        # qT prefetch: lands during groups 2-3, needed by B
                        for b in range(B):
                            sl = slice(b * LT, (b + 1) * LT)
                            nc.sync.dma_start(qT_all[:, :, sl], qT_r[:, :, sl])
                    if p == NG - 1:
                        # phase B's first weight pair, so B starts stall-free
                        wq_pre = prep.tile([128, KB, 2 * HD], F8,
                                           name="wq_pre")
                        nc.sync.dma_start(wq_pre[:], wq_r[:, :, 0:2 * HD])
                        wg_pre = prep.tile([128, KB, 2 * HD], F8,
                                           name="wg_pre")
                        nc.sync.dma_start(wg_pre[:], wg_r[:, :, 0:2 * HD])

                    def emit_summary(ent):
                        b_, mm, kt_, vt_, acc_ = ent
                        for h2 in range(HPG):
                            nc.tensor.matmul(
                                acc_[h2][:, 0:129],
                                kt_[:, h2 * HD:(h2 + 1) * HD],
                                vt_[:, h2, :],
                                start=(mm == 0), stop=(mm == MB - 1))
                        if mm == MB - 1:
                            kvs_sb = sba.tile([128, HPG, 129], F32, tag="kvs",
                                              bufs=2, name=f"kvs{p}_{b_}")
                            for h2 in range(HPG):
                                nc.vector.tensor_copy(kvs_sb[:, h2, :],
                                                      acc_[h2][:, 0:129])
                            nc.sync.dma_start(ar_in[p][b_], kvs_sb[:])

                    pend = []
                    for b in range(B):
                        # one full PSUM bank per head accumulator: a start=True
                        # matmul clears has_written for its whole bank, so
                        # accumulation groups must never share a bank
                        acc = [psa.tile([128, 512], F32, tag=f"acc{j}",
                                        bufs=1, name=f"acc{p}_{b}_{j}")
                               for j in range(HPG)]
                        for m in range(MB):
                            kp = psa.tile([128, WG], F32, tag="kp", bufs=2,
                                          name=f"kp{p}_{b}_{m}")
                            vp = psa.tile([128, HPG, HD], F32, tag="vp", bufs=2,
                                          name=f"vp{p}_{b}_{m}")
                            ksl = slice(b * S_LOC + m * 128,
                                        b * S_LOC + (m + 1) * 128)
                            for k in range(KH):
                                nc.tensor.matmul(kp[:],
                                                 kvT8_all[:, 2 * k:2 * k + 2, ksl],
                                                 wkv_k[:, 2 * k:2 * k + 2, :],
                                                 start=(k == 0), stop=(k == KH - 1),
                                                 perf_mode=DR)
                            for k in range(KB):
                                nc.tensor.matmul(vp[:], kvT_all[:, k, ksl],
                                                 wkv_v[:, k, :],
                                                 start=(k == 0), stop=(k == KB - 1))
                            # kp = WS*k_pre; compute WS*fm(k_pre) directly --
                            # the WS factor cancels in attn = (q.kvs)/(q.ksum)
                            e_sb = sba.tile([128, WG], F32, tag="e", bufs=2,
                                            name=f"e{p}_{b}_{m}")
                            nc.scalar.activation(e_sb[:], kp[:], AF.Exp,
                                                 scale=1.0 / WS, bias=lnws_sb[:])
                            nc.vector.tensor_scalar_min(e_sb[:], e_sb[:], WS)
                            k_t = sba.tile([128, WG], BF16, tag="kt", bufs=2,
                                           name=f"kt{p}_{b}_{m}")
                            nc.vector.scalar_tensor_tensor(
                                k_t[:], kp[:], WS, e_sb[:], ALU.add, ALU.max)
                            v_t = sba.tile([128, HPG, 129], BF16, tag="vt",
                                           bufs=2, name=f"vt{p}_{b}_{m}")
                            nc.vector.tensor_copy(v_t[:, :, 0:HD], vp[:])
                            nc.vector.memset(v_t[:, :, HD:129], 1.0)
                            # summaries lag one (b, m)-step (crossing batch
                            # boundaries) so PE never waits on evac
                            pend.append((b, m, k_t, v_t, acc))
                            if len(pend) > 1:
                                emit_summary(pend.pop(0))
                    while pend:
                        emit_summary(pend.pop(0))
                    if use_collective:
                        nc.gpsimd.collective_compute(
                            "AllReduce", ALU.add,
                            replica_groups=[list(range(n_cores))],
                            ins=[ar_in[p].opt()], outs=[ar_out[p].opt()])
                    else:
                        nc.sync.dma_start(ar_out[p][:], ar_in[p][:])

            agf_all, _agf_free = tc.tile([128, H, B * LT], BF16,
                                         name="agf_all")

            # ---------------- Phase B: q/gate proj + attention -------------
            prec = tc.alloc_tile_pool(name="preC", bufs=1)
            with tc.tile_pool(name="sbB", bufs=2) as sbb, \
                 tc.tile_pool(name="psB", bufs=2, space="PSUM") as psb:

                def load_w2(j):
                    wq2 = sbb.tile([128, KB, 2 * HD], F8, tag="wq2", bufs=2,
                                   name=f"wq2_{j}")
                    nc.sync.dma_start(wq2[:], wq_r[:, :, j * HD:(j + 2) * HD])
                    wg2 = sbb.tile([128, KB, 2 * HD], F8, tag="wg2", bufs=2,
                                   name=f"wg2_{j}")
                    nc.sync.dma_start(wg2[:], wg_r[:, :, j * HD:(j + 2) * HD])
                    return wq2, wg2

                pend_att = []

                def emit_att(ent):
                    hh_, b_, lsl_, kvs_bf_, ksbc_, qfm_, gate_ = ent
                    att_ps = psb.tile([128, LT], F32, tag="att_ps", bufs=2,
                                      name=f"att_ps_{hh_}_{b_}")
                    nc.tensor.matmul(att_ps[:], kvs_bf_[:, 0:128], qfm_[:],
                                     start=True, stop=True)
                    # den pre-broadcast over partitions: stationary column
                    # j is ksum for every j
                    den_ps = psb.tile([128, LT], F32, tag="den_ps", bufs=2,
                                      name=f"den_ps_{hh_}_{b_}")
                    nc.tensor.matmul(den_ps[:], ksbc_[:], qfm_[:],
                                     start=True, stop=True)
                    rden = sbb.tile([128, LT], BF16, tag="rden", bufs=2,
                                    name=f"rden_{hh_}_{b_}")
                    nc.vector.reciprocal(rden[:], den_ps[:])
                    g2_sb = sbb.tile([128, LT], BF16, tag="g2_sb", bufs=2,
                                     name=f"g2_sb_{hh_}_{b_}")
                    nc.vector.tensor_tensor(g2_sb[:], gate_[:], rden[:],
                                            ALU.mult)
                    nc.vector.tensor_tensor(agf_all[:, hh_, lsl_], att_ps[:],
                                            g2_sb[:], ALU.mult)

                cur_w = (wq_pre, wg_pre)
                for h in range(H):
                    p, hh = divmod(h, HPG)
                    if h % 2 == 0:
                        if h > 0:
                            cur_w = nxt_w
                        nxt_w = load_w2(h + 2) if h + 2 < H else None
                    if h == H - 1:
                        # phase C's first weight pair
                        wo_pre = prec.tile([128, KB, 2 * HD], BF16,
                                           name="wo_pre")
                        nc.sync.dma_start(wo_pre[:], wo_r[:, :, 0:2 * HD])
                    wq_h = cur_w[0][:, :, (h % 2) * HD:(h % 2 + 1) * HD]
                    wg_h = cur_w[1][:, :, (h % 2) * HD:(h % 2 + 1) * HD]
                    bg_h = sbb.tile([128, 1], F32, tag="bg", bufs=2,
                                    name=f"bg{h}")
                    nc.sync.dma_start(bg_h[:], bg_d.ap()[h * HD:(h + 1) * HD, :])
                    for b in range(B):
                        kvs_f = sbb.tile([128, 129], F32, tag="kvsf", bufs=3,
                                         name=f"kvsf{h}_{b}")
                        nc.sync.dma_start(kvs_f[:], ar_out[p][b][:, hh, :])
                        kvs_bf = sbb.tile([128, 129], BF16, tag="kvsbf", bufs=3,
                                          name=f"kvsbf{h}_{b}")
                        nc.vector.tensor_copy(kvs_bf[:], kvs_f[:])
                        ksbc = sbb.tile([128, 128], BF16, tag="ksbc", bufs=3,
                                        name=f"ksbc{h}_{b}")
                        nc.vector.tensor_scalar_mul(ksbc[:], ones_bf[:],
                                                    kvs_f[:, 128:129])

                        lsl = slice(b * LT, (b + 1) * LT)
                        q_ps = psb.tile([128, LT], F32, tag="q_ps", bufs=2,
                                        name=f"q_ps_{h}_{b}")
                        for k in range(KH):
                            nc.tensor.matmul(q_ps[:],
                                             wq_h[:, 2 * k:2 * k + 2, :],
                                             qT_all[:, 2 * k:2 * k + 2, lsl],
                                             start=(k == 0), stop=(k == KH - 1),
                                             perf_mode=DR)
                        # q_ps = WS*q_pre -> qfm = WS*fm(q_pre); the WS factor
                        # cancels between numerator and denominator
                        e2_sb = sbb.tile([128, LT], F32, tag="e2_sb", bufs=2,
                                         name=f"e2_sb_{h}_{b}")
                        nc.scalar.activation(e2_sb[:], q_ps[:], AF.Exp,
                                             scale=1.0 / WS, bias=lnws_sb[:])
                        nc.vector.tensor_scalar_min(e2_sb[:], e2_sb[:], WS)
                        qfm = sbb.tile([128, LT], BF16, tag="qfm", bufs=2,
                                       name=f"qfm_{h}_{b}")
                        nc.vector.scalar_tensor_tensor(
                            qfm[:], q_ps[:], WS, e2_sb[:], ALU.add, ALU.max)

                        g_ps = psb.tile([128, LT], F32, tag="g_ps", bufs=2,
                                        name=f"g_ps_{h}_{b}")
                        for k in range(KH):
                            nc.tensor.matmul(g_ps[:],
                                             wg_h[:, 2 * k:2 * k + 2, :],
                                             qT_all[:, 2 * k:2 * k + 2, lsl],
                                             start=(k == 0), stop=(k == KH - 1),
                                             perf_mode=DR)
                        gate_sb = sbb.tile([128, LT], BF16, tag="gate_sb",
                                           bufs=2, name=f"gate_sb_{h}_{b}")
                        nc.scalar.activation(gate_sb[:], g_ps[:], AF.Sigmoid,
                                             bias=bg_h[:], scale=1.0 / WS)

                        # att/den lag one (h, b)-step (like phase A's summary
                        # lag) so the PE never waits on the qfm ACT/DVE tail
                        pend_att.append((h, b, lsl, kvs_bf, ksbc, qfm,
                                         gate_sb))
                        if len(pend_att) > 1:
                            emit_att(pend_att.pop(0))
                if pend_att:
                    emit_att(pend_att.pop(0))

            # ---------------- Phase C: output projection -------------------
            with tc.tile_pool(name="sbC", bufs=2) as sbc, \
                 tc.tile_pool(name="psC", bufs=2, space="PSUM") as psc:

                def load_wo2(j):
                    wo2 = sbc.tile([128, KB, 2 * HD], BF16, tag="wo2", bufs=2,
                                   name=f"wo2_{j}")
                    nc.sync.dma_start(wo2[:], wo_r[:, :, j * HD:(j + 2) * HD])
                    return wo2

                cur_wo = wo_pre
                for do in range(KB):
                    if do % 2 == 0:
                        if do > 0:
                            cur_wo = nxt_wo
                        nxt_wo = load_wo2(do + 2) if do + 2 < KB else None
                    wo_do = cur_wo[:, :, (do % 2) * HD:(do % 2 + 1) * HD]
                    # hh-outer with all 4 batches inner: each stationary
                    # weight block is loaded once instead of 4 times
                    o_ps = [psc.tile([128, LT], F32, tag=f"o_ps{b}", bufs=1,
                                     name=f"o_ps_{do}_{b}") for b in range(B)]
                    for hh in range(H):
                        for b in range(B):
                            lsl = slice(b * LT, (b + 1) * LT)
                            nc.tensor.matmul(o_ps[b][:], wo_do[:, hh, :],
                                             agf_all[:, hh, lsl],
                                             start=(hh == 0),
                                             stop=(hh == H - 1))
                    for b in range(B):
                        lsl = slice(b * LT, (b + 1) * LT)
                        ot_sb = sbc.tile([128, LT], BF16, tag="ot_sb", bufs=4,
                                         name=f"ot_sb_{do}_{b}")
                        nc.scalar.copy(ot_sb[:], o_ps[b][:])
                        nc.sync.dma_start(
                            outT.ap()[do * 128:(do + 1) * 128, lsl], ot_sb[:])

            prec.release()
            _agf_free()

    nc.compile()
    return nc


def _to_f8(x):
    return np.clip(np.asarray(x, np.float32), -240.0, 240.0).astype(
        ml_dtypes.float8_e4m3)


def prep_in_maps(query, kv, Wq, Wg, bg, Wkv, Wo, n_cores=N_CORES):
    B, L, D = query.shape
    S = kv.shape[1]
    LT = L // n_cores
    S_LOC = S // n_cores
    bf = ml_dtypes.bfloat16

    Wkv = np.asarray(Wkv, np.float32)
    wq_t = _to_f8(np.ascontiguousarray(np.asarray(Wq).T) * WS)
    wg_t = _to_f8(np.ascontiguousarray(np.asarray(Wg).T) * WS)
    wkvk_t = _to_f8(np.ascontiguousarray(Wkv[:D].T) * WS)
    wkvv_t = np.ascontiguousarray(Wkv[D:].T).astype(bf)
    wo_t = np.ascontiguousarray(np.asarray(Wo).T).astype(bf)
    bg_d = np.ascontiguousarray(np.asarray(bg, dtype=np.float32).reshape(D, 1))
    query = np.asarray(query)
    kv = np.asarray(kv)

    in_maps = []
    for c in range(n_cores):
        qs = query[:, c * LT:(c + 1) * LT, :]          # [B, LT, D]
        qT_c = _to_f8(np.ascontiguousarray(
            qs.transpose(2, 0, 1).reshape(D, B * LT)))
        ks = kv[:, c * S_LOC:(c + 1) * S_LOC, :]       # [B, S_LOC, D]
        kvT_full = np.ascontiguousarray(
            ks.transpose(2, 0, 1).reshape(D, B * S_LOC))
        kvT_c = kvT_full.astype(bf)
        kvT8_c = _to_f8(kvT_full)
        in_maps.append({
            "qT": qT_c, "kvT": kvT_c, "kvT8": kvT8_c,
            "wq_t": wq_t, "wg_t": wg_t, "wkvk_t": wkvk_t, "wkvv_t": wkvv_t,
            "wo_t": wo_t, "bg_d": bg_d,
        })
    return in_maps


def assemble_output(results, B, L, D, n_cores=N_CORES):
    LT = L // n_cores
    out = np.empty((B, L, D), np.float32)
    for c in range(n_cores):
        outT = np.asarray(results[c]["outT"]).astype(np.float32)  # [D, B*LT]
        per = outT.reshape(D, B, LT)
        out[:, c * LT:(c + 1) * LT, :] = per.transpose(1, 2, 0)
    return out


_NC_CACHE = {}


def _get_module(key):
    if key not in _NC_CACHE:
        B, L, S, D, H = key
        _NC_CACHE[key] = build_module(B=B, L=L, S=S, D=D, H=H)
    return _NC_CACHE[key]


def kernel(query, kv, Wq, Wg, bg, Wkv, Wo):
    query = np.asarray(query)
    kv = np.asarray(kv)
    B, L, D = query.shape
    S = kv.shape[1]
    H = 16
    nc = _get_module((B, L, S, D, H))
    in_maps = prep_in_maps(query, kv, Wq, Wg, bg, Wkv, Wo)
    res = run_bass_kernel_spmd(nc, in_maps, core_ids=list(range(N_CORES)))
    return assemble_output(res.results, B, L, D)


# revision 21
# speedup vs baseline: 1.0831x; 1.0831x over previous
"""Multi-head linear cross-attention Trainium2 kernel (8-core SPMD).

Math (reference):
    q    = fm(query @ Wq.T)        fm(x) = elu(x)+1 = max(x+1, min(exp(x), 1))
    gate = sigmoid(query @ Wg.T + bg)
    k, v = split(kv @ Wkv.T); k = fm(k)
    kvs  = k^T v per (b, h)   [hd, hd];  ksum = k^T 1  [hd]
    attn = (q @ kvs) / (q . ksum + eps)
    out  = (attn * gate) @ Wo.T

Sharding: kv-side split along S (each core takes S/8 rows of every batch,
computes partial [k^T v | k^T 1] summaries, AllReduce over the 8 cores);
q-side split along L (each core takes L/8 rows of every batch).

Precision: the q, gate and k projections run as fp8e4 DoubleRow
matmuls (2 fp8 contractions per PE cell per cycle -> up to 2x matmul
rate). Their weights are host-scaled by WS=64 to keep fp8 operands out
of the subnormal range; the compensation folds away algebraically:
  - the q/k feature maps are computed at 64x scale (exp(x/64 + ln64)),
    and the 64x factors cancel exactly in attn = (q.kvs)/(q.ksum).
  - gate uses activation scale=1/64 before the sigmoid.
The v projection, summaries, attention and output projection stay bf16:
their quantization error would hit the output directly (measured ~3.8e-2
each in simulation vs the 1.2e-2 this mix achieves).

Phase A streams Wkv once in 4 head-groups (k-block + v-block per group),
holding kvT resident (fp8 copy for the k matmuls, bf16 for v); summaries
are computed per (b, m)-tile right after projection (with [v | 1] fused
so k^T*1 rides the same matmul) and each group's summaries are
AllReduced as soon as they are ready, so phase B's head loop never waits
on a collective. den is computed pre-broadcast with a ksum-replicated
stationary matmul.

Layout: all activations are feature-major ([d, token]) on device; the
host pre-transposes/casts inputs and re-transposes the fp32 output.
"""

import numpy as np
import ml_dtypes

import concourse.bacc as bacc
import concourse.mybir as mybir
import concourse.tile as tile
from concourse.bass_utils import run_bass_kernel_spmd

BF16 = mybir.dt.bfloat16
F8 = mybir.dt.float8e4
F32 = mybir.dt.float32
AF = mybir.ActivationFunctionType
ALU = mybir.AluOpType
DR = mybir.MatmulPerfMode.DoubleRow

N_CORES = 8
WS = 64.0          # fp8 weight scale
LN_WS = float(np.log(WS))


def build_module(B=4, L=4096, S=4096, D=2048, H=16, n_cores=N_CORES,
                 use_collective=True):
    """Emit the SPMD bass module. Requires D // H == 128."""
    HD = D // H
    assert HD == 128
    KB = D // 128            # d_in 128-blocks
    KH = KB // 2             # d_in 256-blocks (fp8 DoubleRow steps)
    S_LOC = S // n_cores     # s rows per batch per core
    MB = S_LOC // 128        # s-tiles per batch
    LT = L // n_cores        # l cols per batch per core (free dim of q-side matmuls)
    NG = 4                   # head groups for wkv streaming
    HPG = H // NG            # heads per group
    WG = HPG * HD            # wkv columns per group block (512)
    assert S_LOC % 128 == 0 and D % 128 == 0 and H % NG == 0
    assert LT <= 512 and WG <= 512

    nc = bacc.Bacc("TRN2", target_bir_lowering=False, debug=False,
                   num_devices=n_cores)

    qT = nc.dram_tensor("qT", [D, B * LT], F8, kind="ExternalInput")
    kvT = nc.dram_tensor("kvT", [D, B * S_LOC], BF16, kind="ExternalInput")
    kvT8 = nc.dram_tensor("kvT8", [D, B * S_LOC], F8, kind="ExternalInput")
    wq_t = nc.dram_tensor("wq_t", [D, D], F8, kind="ExternalInput")
    wg_t = nc.dram_tensor("wg_t", [D, D], F8, kind="ExternalInput")
    wkvk_t = nc.dram_tensor("wkvk_t", [D, D], F8, kind="ExternalInput")
    wkvv_t = nc.dram_tensor("wkvv_t", [D, D], BF16, kind="ExternalInput")
    wo_t = nc.dram_tensor("wo_t", [D, D], BF16, kind="ExternalInput")
    bg_d = nc.dram_tensor("bg_d", [D, 1], F32, kind="ExternalInput")
    outT = nc.dram_tensor("outT", [D, B * LT], BF16, kind="ExternalOutput")

    qT_r = qT.ap().rearrange("(k p) l -> p k l", p=128)
    kvT_r = kvT.ap().rearrange("(k p) s -> p k s", p=128)
    kvT8_r = kvT8.ap().rearrange("(k p) s -> p k s", p=128)
    wq_r = wq_t.ap().rearrange("(k p) f -> p k f", p=128)
    wg_r = wg_t.ap().rearrange("(k p) f -> p k f", p=128)
    wkvk_r = wkvk_t.ap().rearrange("(k p) f -> p k f", p=128)
    wkvv_r = wkvv_t.ap().rearrange("(k p) f -> p k f", p=128)
    wo_r = wo_t.ap().rearrange("(k p) f -> p k f", p=128)

    with nc.allow_low_precision(reason="bf16/fp8 matmul kernel"), \
         tile.TileContext(nc) as tc:
        with tc.tile_pool(name="const", bufs=1) as constp, \
             tc.tile_pool(name="qt", bufs=1) as qtp, \
             tc.tile_pool(name="pre", bufs=1) as prep, \
             tc.tile_pool(name="dram", bufs=1, space="DRAM") as dr:

            ones_bf = constp.tile([128, 128], BF16)
            nc.vector.memset(ones_bf[:], 1.0)
            lnws_sb = constp.tile([128, 1], F32)
            nc.vector.memset(lnws_sb[:], LN_WS)

            qT_all = qtp.tile([128, KB, B * LT], F8)

            ar_in = [dr.tile([B, 128, HPG, 129], F32, name=f"ar_in{p}")
                     for p in range(NG)]
            ar_out = [dr.tile([B, 128, HPG, 129], F32, addr_space="Shared",
                              name=f"ar_out{p}") for p in range(NG)]

            # ---------------- Phase A: kv projection + summaries ----------
            with tc.tile_pool(name="kvt", bufs=1) as kvtp, \
                 tc.tile_pool(name="sbA", bufs=2) as sba, \
                 tc.tile_pool(name="psA", bufs=2, space="PSUM") as psa:
                kvT_all = kvtp.tile([128, KB, B * S_LOC], BF16)
                kvT8_all = kvtp.tile([128, KB, B * S_LOC], F8)
                wkv_vs = {}

                def load_wkv_v(p):
                    wkv_vs[p] = sba.tile([128, KB, WG], BF16, tag="wkv_v",
                                         bufs=2, name=f"wkvv{p}")
                    nc.sync.dma_start(wkv_vs[p][:],
                                      wkvv_r[:, :, p * WG:(p + 1) * WG])

                for p in range(NG):
                    wkv_k = sba.tile([128, KB, WG], F8, tag="wkv_k", bufs=2,
                                     name=f"wkvk{p}")
                    nc.sync.dma_start(wkv_k[:],
                                      wkvk_r[:, :, p * WG:(p + 1) * WG])
                    if p == 0:
                        # dependency-ordered preload: everything the first
                        # (b=0, m=0) tile needs first, the bulk afterwards
                        nc.sync.dma_start(kvT8_all[:, :, 0:128],
                                          kvT8_r[:, :, 0:128])
                        load_wkv_v(0)
                        nc.sync.dma_start(kvT_all[:, :, 0:128],
                                          kvT_r[:, :, 0:128])
                        if S_LOC > 128:
                            nc.sync.dma_start(kvT8_all[:, :, 128:S_LOC],
                                              kvT8_r[:, :, 128:S_LOC])
                            nc.sync.dma_start(kvT_all[:, :, 128:S_LOC],
                                              kvT_r[:, :, 128:S_LOC])
                        for b in range(1, B):
                            sl = slice(b * S_LOC, (b + 1) * S_LOC)
                            nc.sync.dma_start(kvT8_all[:, :, sl], kvT8_r[:, :, sl])
                            nc.sync.dma_start(kvT_all[:, :, sl], kvT_r[:, :, sl])
                    if p + 1 < NG:
                        load_wkv_v(p + 1)  # prefetch next group's v-weights
                    wkv_v = wkv_vs[p]
                    if p == 2:
                        # qT prefetch: lands during groups 2-3, needed by B
                        for b in range(B):
                            sl = slice(b * LT, (b + 1) * LT)
                            nc.sync.dma_start(qT_all[:, :, sl], qT_r[:, :, sl])
                    if p == NG - 1:
                        # phase B's first weight pair, so B starts stall-free
                        wq_pre = prep.tile([128, KB, 2 * HD], F8,
                                           name="wq_pre")
                        nc.sync.dma_start(wq_pre[:], wq_r[:, :, 0:2 * HD])
                        wg_pre = prep.tile([128, KB, 2 * HD], F8,
                                           name="wg_pre")
                        nc.sync.dma_start(wg_pre[:], wg_r[:, :, 0:2 * HD])

                    def emit_summary(ent):
                        b_, mm, kt_, vt_, acc_ = ent
                        for h2 in range(HPG):
                            nc.tensor.matmul(
                                acc_[h2][:, 0:129],
                                kt_[:, h2 * HD:(h2 + 1) * HD],
                                vt_[:, h2, :],
                                start=(mm == 0), stop=(mm == MB - 1))
                        if mm == MB - 1:
                            kvs_sb = sba.tile([128, HPG, 129], F32, tag="kvs",
                                              bufs=2, name=f"kvs{p}_{b_}")
                            for h2 in range(HPG):
                                nc.vector.tensor_copy(kvs_sb[:, h2, :],
                                                      acc_[h2][:, 0:129])
                            nc.sync.dma_start(ar_in[p][b_], kvs_sb[:])

                    pend = []
                    for b in range(B):
                        # one full PSUM bank per head accumulator: a start=True
                        # matmul clears has_written for its whole bank, so
                        # accumulation groups must never share a bank
                        acc = [psa.tile([128, 512], F32, tag=f"acc{j}",
                                        bufs=1, name=f"acc{p}_{b}_{j}")
                               for j in range(HPG)]
                        for m in range(MB):
                            kp = psa.tile([128, WG], F32, tag="kp", bufs=2,
                                          name=f"kp{p}_{b}_{m}")
                            vp = psa.tile([128, HPG, HD], F32, tag="vp", bufs=2,
                                          name=f"vp{p}_{b}_{m}")
                            ksl = slice(b * S_LOC + m * 128,
                                        b * S_LOC + (m + 1) * 128)
                            for k in range(KH):
                                nc.tensor.matmul(kp[:],
                                                 kvT8_all[:, 2 * k:2 * k + 2, ksl],
                                                 wkv_k[:, 2 * k:2 * k + 2, :],
                                                 start=(k == 0), stop=(k == KH - 1),
                                                 perf_mode=DR)
                            for k in range(KB):
                                nc.tensor.matmul(vp[:], kvT_all[:, k, ksl],
                                                 wkv_v[:, k, :],
                                                 start=(k == 0), stop=(k == KB - 1))
                            # kp = WS*k_pre; compute WS*fm(k_pre) directly --
                            # the WS factor cancels in attn = (q.kvs)/(q.ksum)
                            e_sb = sba.tile([128, WG], F32, tag="e", bufs=2,
                                            name=f"e{p}_{b}_{m}")
                            nc.scalar.activation(e_sb[:], kp[:], AF.Exp,
                                                 scale=1.0 / WS, bias=lnws_sb[:])
                            nc.vector.tensor_scalar_min(e_sb[:], e_sb[:], WS)
                            k_t = sba.tile([128, WG], BF16, tag="kt", bufs=2,
                                           name=f"kt{p}_{b}_{m}")
                            nc.vector.scalar_tensor_tensor(
                                k_t[:], kp[:], WS, e_sb[:], ALU.add, ALU.max)
                            v_t = sba.tile([128, HPG, 129], BF16, tag="vt",
                                           bufs=2, name=f"vt{p}_{b}_{m}")
                            nc.vector.tensor_copy(v_t[:, :, 0:HD], vp[:])
                            nc.vector.memset(v_t[:, :, HD:129], 1.0)
                            # summaries lag one (b, m)-step (crossing batch
                            # boundaries) so PE never waits on evac
                            pend.append((b, m, k_t, v_t, acc))
                            if len(pend) > 1:
                                emit_summary(pend.pop(0))
                    while pend:
                        emit_summary(pend.pop(0))
                    if use_collective:
                        nc.gpsimd.collective_compute(
                            "AllReduce", ALU.add,
                            replica_groups=[list(range(n_cores))],
                            ins=[ar_in[p].opt()], outs=[ar_out[p].opt()])
                    else:
                        nc.sync.dma_start(ar_out[p][:], ar_in[p][:])

            agf_all, _agf_free = tc.tile([128, H, B * LT], BF16,
                                         name="agf_all")

            # ---------------- Phase B: q/gate proj + attention -------------
            prec = tc.alloc_tile_pool(name="preC", bufs=1)
            with tc.tile_pool(name="sbB", bufs=2) as sbb, \
                 tc.tile_pool(name="psB", bufs=2, space="PSUM") as psb:

                def load_w2(j):
                    wq2 = sbb.tile([128, KB, 2 * HD], F8, tag="wq2", bufs=2,
                                   name=f"wq2_{j}")
                    nc.sync.dma_start(wq2[:], wq_r[:, :, j * HD:(j + 2) * HD])
                    wg2 = sbb.tile([128, KB, 2 * HD], F8, tag="wg2", bufs=2,
                                   name=f"wg2_{j}")
                    nc.sync.dma_start(wg2[:], wg_r[:, :, j * HD:(j + 2) * HD])
                    return wq2, wg2

                cur_w = (wq_pre, wg_pre)
                for h in range(H):
                    p, hh = divmod(h, HPG)
                    if h % 2 == 0:
                        if h > 0:
                            cur_w = nxt_w
                        nxt_w = load_w2(h + 2) if h + 2 < H else None
                    if h == H - 1:
                        # phase C's first weight pair
                        wo_pre = prec.tile([128, KB, 2 * HD], BF16,
                                           name="wo_pre")
                        nc.sync.dma_start(wo_pre[:], wo_r[:, :, 0:2 * HD])
                    wq_h = cur_w[0][:, :, (h % 2) * HD:(h % 2 + 1) * HD]
                    wg_h = cur_w[1][:, :, (h % 2) * HD:(h % 2 + 1) * HD]
                    bg_h = sbb.tile([128, 1], F32, tag="bg", bufs=2,
                                    name=f"bg{h}")
                    nc.sync.dma_start(bg_h[:], bg_d.ap()[h * HD:(h + 1) * HD, :])
                    for b in range(B):
                        kvs_f = sbb.tile([128, 129], F32, tag="kvsf", bufs=3,
                                         name=f"kvsf{h}_{b}")
                        nc.sync.dma_start(kvs_f[:], ar_out[p][b][:, hh, :])
                        kvs_bf = sbb.tile([128, 129], BF16, tag="kvsbf", bufs=3,
                                          name=f"kvsbf{h}_{b}")
                        nc.vector.tensor_copy(kvs_bf[:], kvs_f[:])
                        ksbc = sbb.tile([128, 128], BF16, tag="ksbc", bufs=3,
                                        name=f"ksbc{h}_{b}")
                        nc.vector.tensor_scalar_mul(ksbc[:], ones_bf[:],
                                                    kvs_f[:, 128:129])

                        lsl = slice(b * LT, (b + 1) * LT)
                        q_ps = psb.tile([128, LT], F32, tag="q_ps", bufs=2,
                                        name=f"q_ps_{h}_{b}")
                        for k in range(KH):
                            nc.tensor.matmul(q_ps[:],
                                             wq_h[:, 2 * k:2 * k + 2, :],
                                             qT_all[:, 2 * k:2 * k + 2, lsl],
                                             start=(k == 0), stop=(k == KH - 1),
                                             perf_mode=DR)
                        # q_ps = WS*q_pre -> qfm = WS*fm(q_pre); the WS factor
                        # cancels between numerator and denominator
                        e2_sb = sbb.tile([128, LT], F32, tag="e2_sb", bufs=2,
                                         name=f"e2_sb_{h}_{b}")
                        nc.scalar.activation(e2_sb[:], q_ps[:], AF.Exp,
                                             scale=1.0 / WS, bias=lnws_sb[:])
                        nc.vector.tensor_scalar_min(e2_sb[:], e2_sb[:], WS)
                        qfm = sbb.tile([128, LT], BF16, tag="qfm", bufs=2,
                                       name=f"qfm_{h}_{b}")
                        nc.vector.scalar_tensor_tensor(
                            qfm[:], q_ps[:], WS, e2_sb[:], ALU.add, ALU.max)

                        g_ps = psb.tile([128, LT], F32, tag="g_ps", bufs=2,
                                        name=f"g_ps_{h}_{b}")
                        for k in range(KH):
                            nc.tensor.matmul(g_ps[:],
                                             wg_h[:, 2 * k:2 * k + 2, :],
                                             qT_all[:, 2 * k:2 * k + 2, lsl],
                                             start=(k == 0), stop=(k == KH - 1),
                                             perf_mode=DR)
                        gate_sb = sbb.tile([128, LT], BF16, tag="gate_sb",
                                           bufs=2, name=f"gate_sb_{h}_{b}")
                        nc.scalar.activation(gate_sb[:], g_ps[:], AF.Sigmoid,
                                             bias=bg_h[:], scale=1.0 / WS)

                        att_ps = psb.tile([128, LT], F32, tag="att_ps", bufs=2,
                                          name=f"att_ps_{h}_{b}")
                        nc.tensor.matmul(att_ps[:], kvs_bf[:, 0:128], qfm[:],
                                         start=True, stop=True)
                        # den pre-broadcast over partitions: stationary column
                        # j is ksum for every j
                        den_ps = psb.tile([128, LT], F32, tag="den_ps", bufs=2,
                                          name=f"den_ps_{h}_{b}")
                        nc.tensor.matmul(den_ps[:], ksbc[:], qfm[:],
                                         start=True, stop=True)
                        rden = sbb.tile([128, LT], BF16, tag="rden", bufs=2,
                                        name=f"rden_{h}_{b}")
                        nc.vector.reciprocal(rden[:], den_ps[:])
                        g2_sb = sbb.tile([128, LT], BF16, tag="g2_sb", bufs=2,
                                         name=f"g2_sb_{h}_{b}")
                        nc.vector.tensor_tensor(g2_sb[:], gate_sb[:], rden[:],
                                                ALU.mult)
                        nc.vector.tensor_tensor(agf_all[:, h, lsl], att_ps[:],
                                                g2_sb[:], ALU.mult)

            # ---------------- Phase C: output projection -------------------
            with tc.tile_pool(name="sbC", bufs=2) as sbc, \
                 tc.tile_pool(name="psC", bufs=2, space="PSUM") as psc:

                def load_wo2(j):
                    wo2 = sbc.tile([128, KB, 2 * HD], BF16, tag="wo2", bufs=2,
                                   name=f"wo2_{j}")
                    nc.sync.dma_start(wo2[:], wo_r[:, :, j * HD:(j + 2) * HD])
                    return wo2

                cur_wo = wo_pre
                for do in range(KB):
                    if do % 2 == 0:
                        if do > 0:
                            cur_wo = nxt_wo
                        nxt_wo = load_wo2(do + 2) if do + 2 < KB else None
                    wo_do = cur_wo[:, :, (do % 2) * HD:(do % 2 + 1) * HD]
                    # hh-outer with all 4 batches inner: each stationary
                    # weight block is loaded once instead of 4 times
                    o_ps = [psc.tile([128, LT], F32, tag=f"o_ps{b}", bufs=1,
                                     name=f"o_ps_{do}_{b}") for b in range(B)]
                    for hh in range(H):
                        for b in range(B):
                            lsl = slice(b * LT, (b + 1) * LT)
                            nc.tensor.matmul(o_ps[b][:], wo_do[:, hh, :],
                                             agf_all[:, hh, lsl],
                                             start=(hh == 0),
                                             stop=(hh == H - 1))
                    for b in range(B):
                        lsl = slice(b * LT, (b + 1) * LT)
                        ot_sb = sbc.tile([128, LT], BF16, tag="ot_sb", bufs=4,
                                         name=f"ot_sb_{do}_{b}")
                        nc.scalar.copy(ot_sb[:], o_ps[b][:])
                        nc.sync.dma_start(
                            outT.ap()[do * 128:(do + 1) * 128, lsl], ot_sb[:])

            prec.release()
            _agf_free()

    nc.compile()
    return nc


def _to_f8(x):
    return np.clip(np.asarray(x, np.float32), -240.0, 240.0).astype(
        ml_dtypes.float8_e4m3)


def prep_in_maps(query, kv, Wq, Wg, bg, Wkv, Wo, n_cores=N_CORES):
    B, L, D = query.shape
    S = kv.shape[1]
    LT = L // n_cores
    S_LOC = S // n_cores
    bf = ml_dtypes.bfloat16

    Wkv = np.asarray(Wkv, np.float32)
    wq_t = _to_f8(np.ascontiguousarray(np.asarray(Wq).T) * WS)
    wg_t = _to_f8(np.ascontiguousarray(np.asarray(Wg).T) * WS)
    wkvk_t = _to_f8(np.ascontiguousarray(Wkv[:D].T) * WS)
    wkvv_t = np.ascontiguousarray(Wkv[D:].T).astype(bf)
    wo_t = np.ascontiguousarray(np.asarray(Wo).T).astype(bf)
    bg_d = np.ascontiguousarray(np.asarray(bg, dtype=np.float32).reshape(D, 1))
    query = np.asarray(query)
    kv = np.asarray(kv)

    in_maps = []
    for c in range(n_cores):
        qs = query[:, c * LT:(c + 1) * LT, :]          # [B, LT, D]
        qT_c = _to_f8(np.ascontiguousarray(
            qs.transpose(2, 0, 1).reshape(D, B * LT)))
        ks = kv[:, c * S_LOC:(c + 1) * S_LOC, :]       # [B, S_LOC, D]
        kvT_full = np.ascontiguousarray(
            ks.transpose(2, 0, 1).reshape(D, B * S_LOC))
        kvT_c = kvT_full.astype(bf)
        kvT8_c = _to_f8(kvT_full)
        in_maps.append({
            "qT": qT_c, "kvT": kvT_c, "kvT8": kvT8_c,
            "wq_t": wq_t, "wg_t": wg_t, "wkvk_t": wkvk_t, "wkvv_t": wkvv_t,
            "wo_t": wo_t, "bg_d": bg_d,
        })
    return in_maps


def assemble_output(results, B, L, D, n_cores=N_CORES):
    LT = L // n_cores
    out = np.empty((B, L, D), np.float32)
    for c in range(n_cores):
        outT = np.asarray(results[c]["outT"]).astype(np.float32)  # [D, B*LT]
        per = outT.reshape(D, B, LT)
        out[:, c * LT:(c + 1) * LT, :] = per.transpose(1, 2, 0)
    return out


_NC_CACHE = {}


def _get_module(key):
    if key not in _NC_CACHE:
        B, L, S, D, H = key
        _NC_CACHE[key] = build_module(B=B, L=L, S=S, D=D, H=H)
    return _NC_CACHE[key]


def kernel(query, kv, Wq, Wg, bg, Wkv, Wo):
    query = np.asarray(query)
    kv = np.asarray(kv)
    B, L, D = query.shape
    S = kv.shape[1]
    H = 16
    nc = _get_module((B, L, S, D, H))
    in_maps = prep_in_maps(query, kv, Wq, Wg, bg, Wkv, Wo)
    res = run_bass_kernel_spmd(nc, in_maps, core_ids=list(range(N_CORES)))
    return assemble_output(res.results, B, L, D)


# revision 22
# speedup vs baseline: 6.3691x; 5.8806x over previous
"""Multi-head linear cross-attention Trainium2 kernel (8-core SPMD).

Math (reference):
    q    = fm(query @ Wq.T)        fm(x) = elu(x)+1 = max(x+1, min(exp(x), 1))
    gate = sigmoid(query @ Wg.T + bg)
    k, v = split(kv @ Wkv.T); k = fm(k)
    kvs  = k^T v per (b, h)   [hd, hd];  ksum = k^T 1  [hd]
    attn = (q @ kvs) / (q . ksum + eps)
    out  = (attn * gate) @ Wo.T

Sharding: kv-side split along S (each core takes S/8 rows of every batch,
computes partial [k^T v | k^T 1] summaries, AllReduce over the 8 cores);
q-side split along L (each core takes L/8 rows of every batch).

Precision: the q, gate and k projections run as fp8e4 DoubleRow
matmuls (2 fp8 contractions per PE cell per cycle -> up to 2x matmul
rate). Their weights are host-scaled by WS=64 to keep fp8 operands out
of the subnormal range; the compensation folds away algebraically:
  - the q/k feature maps are computed at 64x scale (exp(x/64 + ln64)),
    and the 64x factors cancel exactly in attn = (q.kvs)/(q.ksum).
  - gate uses activation scale=1/64 before the sigmoid.
The v projection, summaries, attention and output projection stay bf16:
their quantization error would hit the output directly (measured ~3.8e-2
each in simulation vs the 1.2e-2 this mix achieves).

Phase A streams Wkv once in 4 head-groups (k-block + v-block per group),
holding kvT resident (fp8 copy for the k matmuls, bf16 for v); summaries
are computed per (b, m)-tile right after projection (with [v | 1] fused
so k^T*1 rides the same matmul) and each group's summaries are
AllReduced as soon as they are ready, so phase B's head loop never waits
on a collective. den is computed pre-broadcast with a ksum-replicated
stationary matmul.

Layout: all activations are feature-major ([d, token]) on device; the
host pre-transposes/casts inputs and re-transposes the fp32 output.
"""

import numpy as np
import ml_dtypes

import concourse.bacc as bacc
import concourse.mybir as mybir
import concourse.tile as tile
from concourse.bass_utils import run_bass_kernel_spmd

BF16 = mybir.dt.bfloat16
F8 = mybir.dt.float8e4
F32 = mybir.dt.float32
AF = mybir.ActivationFunctionType
ALU = mybir.AluOpType
DR = mybir.MatmulPerfMode.DoubleRow

N_CORES = 8
WS = 64.0          # fp8 weight scale
LN_WS = float(np.log(WS))


def build_module(B=4, L=4096, S=4096, D=2048, H=16, n_cores=N_CORES,
                 use_collective=True):
    """Emit the SPMD bass module. Requires D // H == 128."""
    HD = D // H
    assert HD == 128
    KB = D // 128            # d_in 128-blocks
    KH = KB // 2             # d_in 256-blocks (fp8 DoubleRow steps)
    S_LOC = S // n_cores     # s rows per batch per core
    MB = S_LOC // 128        # s-tiles per batch
    LT = L // n_cores        # l cols per batch per core (free dim of q-side matmuls)
    NG = 4                   # head groups for wkv streaming
    HPG = H // NG            # heads per group
    WG = HPG * HD            # wkv columns per group block (512)
    assert S_LOC % 128 == 0 and D % 128 == 0 and H % NG == 0
    assert LT <= 512 and WG <= 512

    nc = bacc.Bacc("TRN2", target_bir_lowering=False, debug=False,
                   num_devices=n_cores)

    qT = nc.dram_tensor("qT", [D, B * LT], F8, kind="ExternalInput")
    kvT = nc.dram_tensor("kvT", [D, B * S_LOC], BF16, kind="ExternalInput")
    kvT8 = nc.dram_tensor("kvT8", [D, B * S_LOC], F8, kind="ExternalInput")
    wq_t = nc.dram_tensor("wq_t", [D, D], F8, kind="ExternalInput")
    wg_t = nc.dram_tensor("wg_t", [D, D], F8, kind="ExternalInput")
    wkvk_t = nc.dram_tensor("wkvk_t", [D, D], F8, kind="ExternalInput")
    wkvv_t = nc.dram_tensor("wkvv_t", [D, D], BF16, kind="ExternalInput")
    wo_t = nc.dram_tensor("wo_t", [D, D], BF16, kind="ExternalInput")
    bg_d = nc.dram_tensor("bg_d", [D, 1], F32, kind="ExternalInput")
    outT = nc.dram_tensor("outT", [D, B * LT], BF16, kind="ExternalOutput")

    qT_r = qT.ap().rearrange("(k p) l -> p k l", p=128)
    kvT_r = kvT.ap().rearrange("(k p) s -> p k s", p=128)
    kvT8_r = kvT8.ap().rearrange("(k p) s -> p k s", p=128)
    wq_r = wq_t.ap().rearrange("(k p) f -> p k f", p=128)
    wg_r = wg_t.ap().rearrange("(k p) f -> p k f", p=128)
    wkvk_r = wkvk_t.ap().rearrange("(k p) f -> p k f", p=128)
    wkvv_r = wkvv_t.ap().rearrange("(k p) f -> p k f", p=128)
    wo_r = wo_t.ap().rearrange("(k p) f -> p k f", p=128)

    with nc.allow_low_precision(reason="bf16/fp8 matmul kernel"), \
         tile.TileContext(nc) as tc:
        with tc.tile_pool(name="const", bufs=1) as constp, \
             tc.tile_pool(name="qt", bufs=1) as qtp, \
             tc.tile_pool(name="pre", bufs=1) as prep, \
             tc.tile_pool(name="dram", bufs=1, space="DRAM") as dr:

            ones_bf = constp.tile([128, 128], BF16)
            nc.vector.memset(ones_bf[:], 1.0)
            lnws_sb = constp.tile([128, 1], F32)
            nc.vector.memset(lnws_sb[:], LN_WS)

            qT_all = qtp.tile([128, KB, B * LT], F8)

            ar_in = [dr.tile([B, 128, HPG, 129], F32, name=f"ar_in{p}")
                     for p in range(NG)]
            ar_out = [dr.tile([B, 128, HPG, 129], F32, addr_space="Shared",
                              name=f"ar_out{p}") for p in range(NG)]

            # ---------------- Phase A: kv projection + summaries ----------
            with tc.tile_pool(name="kvt", bufs=1) as kvtp, \
                 tc.tile_pool(name="sbA", bufs=2) as sba, \
                 tc.tile_pool(name="psA", bufs=2, space="PSUM") as psa:
                kvT_all = kvtp.tile([128, KB, B * S_LOC], BF16)
                kvT8_all = kvtp.tile([128, KB, B * S_LOC], F8)
                wkv_vs = {}

                def load_wkv_v(p):
                    wkv_vs[p] = sba.tile([128, KB, WG], BF16, tag="wkv_v",
                                         bufs=2, name=f"wkvv{p}")
                    nc.sync.dma_start(wkv_vs[p][:],
                                      wkvv_r[:, :, p * WG:(p + 1) * WG])

                for p in range(NG):
                    wkv_k = sba.tile([128, KB, WG], F8, tag="wkv_k", bufs=2,
                                     name=f"wkvk{p}")
                    nc.sync.dma_start(wkv_k[:],
                                      wkvk_r[:, :, p * WG:(p + 1) * WG])
                    if p == 0:
                        # dependency-ordered preload: everything the first
                        # (b=0, m=0) tile needs first, the bulk afterwards
                        nc.sync.dma_start(kvT8_all[:, :, 0:128],
                                          kvT8_r[:, :, 0:128])
                        load_wkv_v(0)
                        nc.sync.dma_start(kvT_all[:, :, 0:128],
                                          kvT_r[:, :, 0:128])
                        if S_LOC > 128:
                            nc.sync.dma_start(kvT8_all[:, :, 128:S_LOC],
                                              kvT8_r[:, :, 128:S_LOC])
                            nc.sync.dma_start(kvT_all[:, :, 128:S_LOC],
                                              kvT_r[:, :, 128:S_LOC])
                        for b in range(1, B):
                            sl = slice(b * S_LOC, (b + 1) * S_LOC)
                            nc.sync.dma_start(kvT8_all[:, :, sl], kvT8_r[:, :, sl])
                            nc.sync.dma_start(kvT_all[:, :, sl], kvT_r[:, :, sl])
                    if p + 1 < NG:
                        load_wkv_v(p + 1)  # prefetch next group's v-weights
                    wkv_v = wkv_vs[p]
                    if p == 2:
                        # qT prefetch: lands during groups 2-3, needed by B
                        for b in range(B):
                            sl = slice(b * LT, (b + 1) * LT)
                            nc.sync.dma_start(qT_all[:, :, sl], qT_r[:, :, sl])
                    if p == NG - 1:
                        # phase B's first weight pair, so B starts stall-free
                        wq_pre = prep.tile([128, KB, 2 * HD], F8,
                                           name="wq_pre")
                        nc.sync.dma_start(wq_pre[:], wq_r[:, :, 0:2 * HD])
                        wg_pre = prep.tile([128, KB, 2 * HD], F8,
                                           name="wg_pre")
                        nc.sync.dma_start(wg_pre[:], wg_r[:, :, 0:2 * HD])

                    def emit_summary(ent):
                        b_, mm, kt_, vt_, acc_ = ent
                        for h2 in range(HPG):
                            nc.tensor.matmul(
                                acc_[h2][:, 0:129],
                                kt_[:, h2 * HD:(h2 + 1) * HD],
                                vt_[:, h2, :],
                                start=(mm == 0), stop=(mm == MB - 1))
                        if mm == MB - 1:
                            kvs_sb = sba.tile([128, HPG, 129], F32, tag="kvs",
                                              bufs=2, name=f"kvs{p}_{b_}")
                            for h2 in range(HPG):
                                nc.vector.tensor_copy(kvs_sb[:, h2, :],
                                                      acc_[h2][:, 0:129])
                            nc.sync.dma_start(ar_in[p][b_], kvs_sb[:])

                    pend = []
                    for b in range(B):
                        # one full PSUM bank per head accumulator: a start=True
                        # matmul clears has_written for its whole bank, so
                        # accumulation groups must never share a bank
                        acc = [psa.tile([128, 512], F32, tag=f"acc{j}",
                                        bufs=1, name=f"acc{p}_{b}_{j}")
                               for j in range(HPG)]
                        for m in range(MB):
                            kp = psa.tile([128, WG], F32, tag="kp", bufs=2,
                                          name=f"kp{p}_{b}_{m}")
                            vp = psa.tile([128, HPG, HD], F32, tag="vp", bufs=2,
                                          name=f"vp{p}_{b}_{m}")
                            ksl = slice(b * S_LOC + m * 128,
                                        b * S_LOC + (m + 1) * 128)
                            for k in range(KH):
                                nc.tensor.matmul(kp[:],
                                                 kvT8_all[:, 2 * k:2 * k + 2, ksl],
                                                 wkv_k[:, 2 * k:2 * k + 2, :],
                                                 start=(k == 0), stop=(k == KH - 1),
                                                 perf_mode=DR)
                            for k in range(KB):
                                nc.tensor.matmul(vp[:], kvT_all[:, k, ksl],
                                                 wkv_v[:, k, :],
                                                 start=(k == 0), stop=(k == KB - 1))
                            # kp = WS*k_pre; compute WS*fm(k_pre) directly --
                            # the WS factor cancels in attn = (q.kvs)/(q.ksum)
                            e_sb = sba.tile([128, WG], F32, tag="e", bufs=2,
                                            name=f"e{p}_{b}_{m}")
                            nc.scalar.activation(e_sb[:], kp[:], AF.Exp,
                                                 scale=1.0 / WS, bias=lnws_sb[:])
                            nc.vector.tensor_scalar_min(e_sb[:], e_sb[:], WS)
                            k_t = sba.tile([128, WG], BF16, tag="kt", bufs=2,
                                           name=f"kt{p}_{b}_{m}")
                            nc.vector.scalar_tensor_tensor(
                                k_t[:], kp[:], WS, e_sb[:], ALU.add, ALU.max)
                            v_t = sba.tile([128, HPG, 129], BF16, tag="vt",
                                           bufs=2, name=f"vt{p}_{b}_{m}")
                            nc.vector.tensor_copy(v_t[:, :, 0:HD], vp[:])
                            nc.vector.memset(v_t[:, :, HD:129], 1.0)
                            # summaries lag one (b, m)-step (crossing batch
                            # boundaries) so PE never waits on evac
                            pend.append((b, m, k_t, v_t, acc))
                            if len(pend) > 1:
                                emit_summary(pend.pop(0))
                    while pend:
                        emit_summary(pend.pop(0))
                    if use_collective:
                        nc.gpsimd.collective_compute(
                            "AllReduce", ALU.add,
                            replica_groups=[list(range(n_cores))],
                            ins=[ar_in[p].opt()], outs=[ar_out[p].opt()])
                    else:
                        nc.sync.dma_start(ar_out[p][:], ar_in[p][:])

            agf_all, _agf_free = tc.tile([128, H, B * LT], BF16,
                                         name="agf_all")

            # ---------------- Phase B: q/gate proj + attention -------------
            prec = tc.alloc_tile_pool(name="preC", bufs=1)
            with tc.tile_pool(name="sbB", bufs=2) as sbb, \
                 tc.tile_pool(name="psB", bufs=2, space="PSUM") as psb:

                def load_w2(j):
                    wq2 = sbb.tile([128, KB, 2 * HD], F8, tag="wq2", bufs=2,
                                   name=f"wq2_{j}")
                    nc.sync.dma_start(wq2[:], wq_r[:, :, j * HD:(j + 2) * HD])
                    wg2 = sbb.tile([128, KB, 2 * HD], F8, tag="wg2", bufs=2,
                                   name=f"wg2_{j}")
                    nc.sync.dma_start(wg2[:], wg_r[:, :, j * HD:(j + 2) * HD])
                    return wq2, wg2

                cur_w = (wq_pre, wg_pre)
                for h in range(H):
                    p, hh = divmod(h, HPG)
                    if h % 2 == 0:
                        if h > 0:
                            cur_w = nxt_w
                        nxt_w = load_w2(h + 2) if h + 2 < H else None
                    if h == H - 1:
                        # phase C's first weight pair
                        wo_pre = prec.tile([128, KB, 2 * HD], BF16,
                                           name="wo_pre")
                        nc.sync.dma_start(wo_pre[:], wo_r[:, :, 0:2 * HD])
                    wq_h = cur_w[0][:, :, (h % 2) * HD:(h % 2 + 1) * HD]
                    wg_h = cur_w[1][:, :, (h % 2) * HD:(h % 2 + 1) * HD]
                    bg_h = sbb.tile([128, 1], F32, tag="bg", bufs=2,
                                    name=f"bg{h}")
                    nc.sync.dma_start(bg_h[:], bg_d.ap()[h * HD:(h + 1) * HD, :])
                    for b in range(B):
                        kvs_f = sbb.tile([128, 129], F32, tag="kvsf", bufs=3,
                                         name=f"kvsf{h}_{b}")
                        nc.sync.dma_start(kvs_f[:], ar_out[p][b][:, hh, :])
                        kvs_bf = sbb.tile([128, 129], BF16, tag="kvsbf", bufs=3,
                                          name=f"kvsbf{h}_{b}")
                        nc.vector.tensor_copy(kvs_bf[:], kvs_f[:])
                        ksbc = sbb.tile([128, 128], BF16, tag="ksbc", bufs=3,
                                        name=f"ksbc{h}_{b}")
                        nc.vector.tensor_scalar_mul(ksbc[:], ones_bf[:],
                                                    kvs_f[:, 128:129])

                        lsl = slice(b * LT, (b + 1) * LT)
                        q_ps = psb.tile([128, LT], F32, tag="q_ps", bufs=2,
                                        name=f"q_ps_{h}_{b}")
                        for k in range(KH):
                            nc.tensor.matmul(q_ps[:],
                                             wq_h[:, 2 * k:2 * k + 2, :],
                                             qT_all[:, 2 * k:2 * k + 2, lsl],
                                             start=(k == 0), stop=(k == KH - 1),
                                             perf_mode=DR)
                        # q_ps = WS*q_pre -> qfm = WS*fm(q_pre); the WS factor
                        # cancels between numerator and denominator
                        e2_sb = sbb.tile([128, LT], F32, tag="e2_sb", bufs=2,
                                         name=f"e2_sb_{h}_{b}")
                        nc.scalar.activation(e2_sb[:], q_ps[:], AF.Exp,
                                             scale=1.0 / WS, bias=lnws_sb[:])
                        nc.vector.tensor_scalar_min(e2_sb[:], e2_sb[:], WS)
                        qfm = sbb.tile([128, LT], BF16, tag="qfm", bufs=2,
                                       name=f"qfm_{h}_{b}")
                        nc.vector.scalar_tensor_tensor(
                            qfm[:], q_ps[:], WS, e2_sb[:], ALU.add, ALU.max)

                        g_ps = psb.tile([128, LT], F32, tag="g_ps", bufs=2,
                                        name=f"g_ps_{h}_{b}")
                        for k in range(KH):
                            nc.tensor.matmul(g_ps[:],
                                             wg_h[:, 2 * k:2 * k + 2, :],
                                             qT_all[:, 2 * k:2 * k + 2, lsl],
                                             start=(k == 0), stop=(k == KH - 1),
                                             perf_mode=DR)
                        gate_sb = sbb.tile([128, LT], BF16, tag="gate_sb",
                                           bufs=2, name=f"gate_sb_{h}_{b}")
                        nc.scalar.activation(gate_sb[:], g_ps[:], AF.Sigmoid,
                                             bias=bg_h[:], scale=1.0 / WS)

                        att_ps = psb.tile([128, LT], F32, tag="att_ps", bufs=2,
                                          name=f"att_ps_{h}_{b}")
                        nc.tensor.matmul(att_ps[:], kvs_bf[:, 0:128], qfm[:],
                                         start=True, stop=True)
                        # den pre-broadcast over partitions: stationary column
                        # j is ksum for every j
                        den_ps = psb.tile([128, LT], F32, tag="den_ps", bufs=2,
                                          name=f"den_ps_{h}_{b}")
                        nc.tensor.matmul(den_ps[:], ksbc[:], qfm[:],
                                         start=True, stop=True)
                        rden = sbb.tile([128, LT], BF16, tag="rden", bufs=2,
                                        name=f"rden_{h}_{b}")
                        nc.vector.reciprocal(rden[:], den_ps[:])
                        g2_sb = sbb.tile([128, LT], BF16, tag="g2_sb", bufs=2,
                                         name=f"g2_sb_{h}_{b}")
                        nc.vector.tensor_tensor(g2_sb[:], gate_sb[:], rden[:],
                                                ALU.mult)
                        nc.vector.tensor_tensor(agf_all[:, h, lsl], att_ps[:],
                                                g2_sb[:], ALU.mult)

            # ---------------- Phase C: output projection -------------------
            with tc.tile_pool(name="sbC", bufs=2) as sbc, \
                 tc.tile_pool(name="psC", bufs=2, space="PSUM") as psc:

                def load_wo2(j):
                    wo2 = sbc.tile([128, KB, 2 * HD], BF16, tag="wo2", bufs=2,
                                   name=f"wo2_{j}")
                    nc.sync.dma_start(wo2[:], wo_r[:, :, j * HD:(j + 2) * HD])
                    return wo2

                cur_wo = wo_pre
                for do in range(KB):
                    if do % 2 == 0:
                        if do > 0:
                            cur_wo = nxt_wo
                        nxt_wo = load_wo2(do + 2) if do + 2 < KB else None
                    wo_do = cur_wo[:, :, (do % 2) * HD:(do % 2 + 1) * HD]
                    for b in range(B):
                        lsl = slice(b * LT, (b + 1) * LT)
                        o_ps = psc.tile([128, LT], F32, tag="o_ps", bufs=2,
                                        name=f"o_ps_{do}_{b}")
                        for hh in range(H):
                            nc.tensor.matmul(o_ps[:], wo_do[:, hh, :],
                                             agf_all[:, hh, lsl],
                                             start=(hh == 0),
                                             stop=(hh == H - 1))
                        ot_sb = sbc.tile([128, LT], BF16, tag="ot_sb", bufs=2,
                                         name=f"ot_sb_{do}_{b}")
                        nc.scalar.copy(ot_sb[:], o_ps[:])
                        nc.sync.dma_start(
                            outT.ap()[do * 128:(do + 1) * 128, lsl], ot_sb[:])

            prec.release()
            _agf_free()

    nc.compile()
    return nc


def _to_f8(x):
    return np.clip(np.asarray(x, np.float32), -240.0, 240.0).astype(
        ml_dtypes.float8_e4m3)


def prep_in_maps(query, kv, Wq, Wg, bg, Wkv, Wo, n_cores=N_CORES):
    B, L, D = query.shape
    S = kv.shape[1]
    LT = L // n_cores
    S_LOC = S // n_cores
    bf = ml_dtypes.bfloat16

    Wkv = np.asarray(Wkv, np.float32)
    wq_t = _to_f8(np.ascontiguousarray(np.asarray(Wq).T) * WS)
    wg_t = _to_f8(np.ascontiguousarray(np.asarray(Wg).T) * WS)
    wkvk_t = _to_f8(np.ascontiguousarray(Wkv[:D].T) * WS)
    wkvv_t = np.ascontiguousarray(Wkv[D:].T).astype(bf)
    wo_t = np.ascontiguousarray(np.asarray(Wo).T).astype(bf)
    bg_d = np.ascontiguousarray(np.asarray(bg, dtype=np.float32).reshape(D, 1))
    query = np.asarray(query)
    kv = np.asarray(kv)

    in_maps = []
    for c in range(n_cores):
        qs = query[:, c * LT:(c + 1) * LT, :]          # [B, LT, D]
        qT_c = _to_f8(np.ascontiguousarray(
            qs.transpose(2, 0, 1).reshape(D, B * LT)))
        ks = kv[:, c * S_LOC:(c + 1) * S_LOC, :]       # [B, S_LOC, D]
        kvT_full = np.ascontiguousarray(
            ks.transpose(2, 0, 1).reshape(D, B * S_LOC))
        kvT_c = kvT_full.astype(bf)
        kvT8_c = _to_f8(kvT_full)
        in_maps.append({
            "qT": qT_c, "kvT": kvT_c, "kvT8": kvT8_c,
            "wq_t": wq_t, "wg_t": wg_t, "wkvk_t": wkvk_t, "wkvv_t": wkvv_t,
            "wo_t": wo_t, "bg_d": bg_d,
        })
    return in_maps


def assemble_output(results, B, L, D, n_cores=N_CORES):
    LT = L // n_cores
    out = np.empty((B, L, D), np.float32)
    for c in range(n_cores):
        outT = np.asarray(results[c]["outT"]).astype(np.float32)  # [D, B*LT]
        per = outT.reshape(D, B, LT)
        out[:, c * LT:(c + 1) * LT, :] = per.transpose(1, 2, 0)
    return out


_NC_CACHE = {}


def _get_module(key):
    if key not in _NC_CACHE:
        B, L, S, D, H = key
        _NC_CACHE[key] = build_module(B=B, L=L, S=S, D=D, H=H)
    return _NC_CACHE[key]


def kernel(query, kv, Wq, Wg, bg, Wkv, Wo):
    query = np.asarray(query)
    kv = np.asarray(kv)
    B, L, D = query.shape
    S = kv.shape[1]
    H = 16
    nc = _get_module((B, L, S, D, H))
    in_maps = prep_in_maps(query, kv, Wq, Wg, bg, Wkv, Wo)
    res = run_bass_kernel_spmd(nc, in_maps, core_ids=list(range(N_CORES)))
    return assemble_output(res.results, B, L, D)
